# revision 1
# baseline (speedup 1.0000x reference)
"""Trainium2 Bass kernel for a RoPE causal-attention layer.

Problem (hardcoded): B=2, T=2048, DIM=1024, H=16 heads, Dh=64, fp32.
  qkv = x @ qkv_w.T + qkv_b ; rope(q), rope(k) ; causal softmax(q k^T / sqrt(Dh)) @ v
  out = ctx @ out_w.T + out_b

Sharding: tensor-parallel over heads — each of the 8 cores owns 2 heads
(qkv_w row-sharded, out_w column-sharded); per-core partial outputs are
summed on the host.

Per-core kernel layout notes:
  * Activations live transposed ([feature, token]) so every matmul
    contracts over partitions: xT [1024, 4096] -> qT/kT/vT [128, 4096].
  * All matmul operands are float32r (fp32 storage, single-pass PE at 4x
    the fp32 matmul rate, ~TF32 operand rounding); PSUM accumulation is
    full fp32.  End-to-end relative error vs the fp32 reference: 2e-4.
  * RoPE pair-interleave is folded into the q/k weight rows on the host
    (de-interleave permutation), making the on-device rotation
    q_rot = q*C + swap_halves(q)*S with contiguous halves; swap_halves is
    one PE permutation matmul, the rest is 3 DVE elementwise ops.
  * Scores are computed TRANSPOSED ([s, t] tiles) and the PV matmul is
    transposed as well: ctx^T(+denominator row) = v_aug^T @ exp(s^T) -
    one K=128/N=512 matmul per s-block, no probability or context
    transposes anywhere, and the context lands pre-transposed for the
    output projection.
  * Softmax denominators come for free from a ones-column appended to V;
    the per-token divide broadcasts the reciprocal denominator row
    across partitions with a tiny K=1 ones-matmul on the PE.
    exp() needs no max-subtraction: inputs are fixed-scale (|score|<~30).
  * Causality: only lower-triangular s-blocks are visited; diagonal
    blocks compute only the at/right-of-diagonal column range and mask
    just the diagonal 128-sub with a [128,128] triangle (exp(-inf)=0
    expressed as a multiplicative 0/1 mask after exp - exact).
  * Phases are software-interleaved at emission time (engines execute
    their instruction streams in order): batch-1 projections overlap
    batch-0 attention, output projection overlaps batch-1 attention; the
    two heads' score/exp/PV chains are interleaved 2-deep so the PE
    always has an independent matmul between a score and its dependent
    PV.
"""

import sys

if "/opt/trn_rl_repo" not in sys.path:
    sys.path.insert(0, "/opt/trn_rl_repo")

import numpy as np

import concourse.bass as bass
import concourse.tile as tile
from concourse import mybir
from concourse.vector_clock import ScopedClock, VectorClock

B, T, DIM = 2, 2048, 1024
H, Dh = 16, 64
NCORES = 8
HPC = H // NCORES          # heads per core
NT = B * T                 # 4096 tokens
RPC = HPC * Dh             # 128 rows per core for each of q/k/v
NQC = NT // 512            # 8 column chunks for projections
SCALE = Dh ** -0.5

F32 = mybir.dt.float32


def _patch_tile_drain():
    """This container's walrus build allows at most ONE semaphore wait per
    instruction (setupSyncWait rejects more).  Two fixes:
      1. Tile's end-of-kernel drain accumulates one wait per live
         semaphore - split into one drain per semaphore.
      2. Any scheduled instruction that received >1 sem waits in stage 1B
         gets its extra waits hoisted onto same-engine NoOps emitted just
         before it during lowering."""
    if getattr(tile.TileContext, "_drain_patched", False):
        return

    def patched(self, tick_clock, wait_clock):
        vec = list(tick_clock.global_clock)
        nz = [(i, t) for i, t in enumerate(vec) if t > 0] or [(0, 0)]
        for i, t in nz:
            cv = [0] * len(vec)
            cv[i] = t
            d = self.nc.sync.drain()
            wait_clock.add_sem_waits(d.ins, ScopedClock({None: VectorClock(cv)}))
        self.nc.all_engine_barrier()
        popped = self.nc._tile_sem_poison_stack.pop()
        assert popped is self._sem_poison
        self.nc.clear_and_free_semaphores(list(self.sems.allocated().values()))
        self.nc.all_engine_barrier()

    tile.TileContext._drain_and_barrier = patched

    orig_cal = tile.TileContext._commit_and_lower

    def patched_cal(self, inst, original_block, old_bb_map, bb_to_exit_bb):
        si = getattr(inst, "sync_info", None)
        eng = getattr(inst, "engine", None)
        if si is not None and si.on_wait and eng in self.nc.engines:
            waits = list(si.on_wait)
            # Matmult can't carry embedded waits at all in this walrus
            # build (fp32/fp32r lowering uses the LDW sync slots); other
            # instructions can carry exactly one.
            keep = 1
            if len(waits) > keep:
                for w in waits[: len(waits) - keep]:
                    nop = self.nc.engines[eng].nop(nofuse=True)
                    nop.ins.sync_info = mybir.SyncInfo(on_wait=[w], on_update=[])
                inst.sync_info = mybir.SyncInfo(
                    on_wait=waits[len(waits) - keep:],
                    on_update=list(si.on_update or []),
                )
        return orig_cal(self, inst, original_block, old_bb_map, bb_to_exit_bb)

    tile.TileContext._commit_and_lower = patched_cal
    tile.TileContext._drain_patched = True


def _rope_tables():
    """C, S [128, 2048] (f32) for the de-interleaved (halves) layout, rows
    duplicated for the 2 heads resident on a core.

    Reference rope on pair i of head_dim: angle_i(t) = t * inv_freq[(2i) % 32];
    de-interleaved row j (j<32: even element of pair j; j>=32: odd element of
    pair j-32):  q_rot = q*C + swap_halves(q)*S with
      C[j]    = cos(angle_{j%32}),  S[j] = -sin(angle_j) if j<32 else +sin(angle_{j-32}).
    """
    half = Dh // 2  # 32
    inv_freq = 1.0 / 10000.0 ** (np.arange(half, dtype=np.float64) / half)
    t = np.arange(T, dtype=np.float64)
    # pair i uses inv_freq[(2i) % 32]
    pair_freq = inv_freq[(2 * np.arange(half)) % half]        # [32]
    ang = np.outer(pair_freq, t)                              # [32, T]
    c32 = np.cos(ang)
    s32 = np.sin(ang)
    C64 = np.concatenate([c32, c32], axis=0)                  # [64, T]
    S64 = np.concatenate([-s32, s32], axis=0)                 # [64, T]
    C = np.concatenate([C64, C64], axis=0).astype(np.float32)  # [128, T]
    S = np.concatenate([S64, S64], axis=0).astype(np.float32)
    return np.ascontiguousarray(C), np.ascontiguousarray(S)


def _deinterleave_perm():
    """perm such that new[d] = old[perm[d]]: halves <- pair-interleaved."""
    p = np.empty(Dh, dtype=np.int64)
    p[: Dh // 2] = 2 * np.arange(Dh // 2)        # evens first
    p[Dh // 2:] = 2 * np.arange(Dh // 2) + 1     # odds second
    return p


def _swap_matrix():
    """[128,128] block-diag permutation: within each head's 64 rows, swap
    the two halves (rows 0..31 <-> 32..63)."""
    P64 = np.zeros((Dh, Dh), dtype=np.float32)
    half = Dh // 2
    P64[np.arange(half), half + np.arange(half)] = 1.0
    P64[half + np.arange(half), np.arange(half)] = 1.0
    M = np.zeros((RPC, RPC), dtype=np.float32)
    M[:Dh, :Dh] = P64
    M[Dh:, Dh:] = P64
    return M


def _tri_masks():
    """masks[k][i, j] = 1 if 128k + i <= j else 0 - the causal pattern of
    transposed-score diagonal blocks [s=128, t=512], k = s-block offset/128
    within the 512-wide t block."""
    i = np.arange(128)[:, None]
    j = np.arange(512)[None, :]
    return np.stack(
        [(128 * k + i <= j).astype(np.float32) for k in range(4)], axis=0
    )


def _build_nc(use_pad_mask: bool, reps: int = 1):
    _patch_tile_drain()
    nc = bass.Bass("TRN2", target_bir_lowering=False, debug=False,
                   num_devices=NCORES)

    # All matmul operands are float32r (same 4-byte storage as f32; the PE
    # runs them single-pass at 4x the fp32 matmul rate, ~TF32 accuracy).
    # PSUM accumulation stays full fp32.
    FR = mybir.dt.float32r

    xT = nc.dram_tensor("xT", [DIM, NT], FR, kind="ExternalInput")
    wq = nc.dram_tensor("wq", [128, DIM], FR, kind="ExternalInput")
    wk = nc.dram_tensor("wk", [128, DIM], FR, kind="ExternalInput")
    wv = nc.dram_tensor("wv", [128, DIM], FR, kind="ExternalInput")
    bq = nc.dram_tensor("bq", [RPC, 1], F32, kind="ExternalInput")
    bk = nc.dram_tensor("bk", [RPC, 1], F32, kind="ExternalInput")
    bv = nc.dram_tensor("bv", [RPC, 1], F32, kind="ExternalInput")
    ropec = nc.dram_tensor("ropec", [RPC, T], FR, kind="ExternalInput")
    ropes = nc.dram_tensor("ropes", [RPC, T], FR, kind="ExternalInput")
    swapm = nc.dram_tensor("swapm", [RPC, RPC], FR, kind="ExternalInput")
    ident = nc.dram_tensor("ident", [128, 128], FR, kind="ExternalInput")
    masks = nc.dram_tensor("masks", [4, 128, 512], FR, kind="ExternalInput")
    wo = nc.dram_tensor("wo", [RPC, DIM], FR, kind="ExternalInput")
    ones64 = nc.dram_tensor("ones64", [1, Dh], FR, kind="ExternalInput")
    if use_pad_mask:
        padv = nc.dram_tensor("padv", [B, 128, T // 128], F32,
                              kind="ExternalInput")
    outT = nc.dram_tensor("outT", [DIM, NT], F32, kind="ExternalOutput")

    EXP = mybir.ActivationFunctionType.Exp
    IDN = mybir.ActivationFunctionType.Identity
    CPY = mybir.ActivationFunctionType.Copy

    with tile.TileContext(nc) as tc:
        with (
            tc.tile_pool(name="consts", bufs=1) as consts,
            tc.tile_pool(name="persist", bufs=1) as persist,
            tc.tile_pool(name="xpool", bufs=14) as xpool,
            tc.tile_pool(name="qkvtmp", bufs=4) as qkvtmp,
            tc.tile_pool(name="ropetmp", bufs=4) as ropetmp,
            tc.tile_pool(name="exppool", bufs=8) as exppool,
            tc.tile_pool(name="normtmp", bufs=4) as normtmp,
            tc.tile_pool(name="outev", bufs=4) as outev,
            tc.tile_pool(name="drampool", bufs=4, space="DRAM") as drampool,
            tc.tile_pool(name="psA", bufs=4, space="PSUM") as psA,
            tc.tile_pool(name="ctxps", bufs=3, space="PSUM") as ctxps,
            tc.tile_pool(name="tps", bufs=1, space="PSUM") as tps,
        ):
            # ---- constants: ordered so Phase A can start ASAP -------------
            # (engines run their DMA streams in program order - weights and
            # rope tables first, the B/C-phase constants last)
            wq_s = consts.tile([128, DIM], FR, tag="wq")
            wk_s = consts.tile([128, DIM], FR, tag="wk")
            wv_s = consts.tile([128, DIM], FR, tag="wv")
            nc.sync.dma_start(out=wq_s[:], in_=wq[:])
            nc.gpsimd.dma_start(out=wk_s[:], in_=wk[:])
            nc.gpsimd.dma_start(out=wv_s[:], in_=wv[:])
            bq_s = consts.tile([RPC, 1], F32, tag="bq")
            bk_s = consts.tile([RPC, 1], F32, tag="bk")
            bv_s = consts.tile([RPC, 1], F32, tag="bv")
            nc.sync.dma_start(out=bq_s[:], in_=bq[:])
            nc.gpsimd.dma_start(out=bk_s[:], in_=bk[:])
            nc.gpsimd.dma_start(out=bv_s[:], in_=bv[:])
            swap_s = consts.tile([RPC, RPC], FR, tag="swapm")
            id_s = consts.tile([128, 128], FR, tag="ident")
            ones_s = consts.tile([1, Dh], FR, tag="ones64")
            nc.gpsimd.dma_start(out=swap_s[:], in_=swapm[:])
            nc.gpsimd.dma_start(out=id_s[:], in_=ident[:])
            nc.gpsimd.dma_start(out=ones_s[:], in_=ones64[:])
            ropec_s = consts.tile([RPC, T], FR, tag="ropec")
            ropes_s = consts.tile([RPC, T], FR, tag="ropes")
            mask_s = consts.tile([128, 4 * 512], FR, tag="masks")
            wo_s = consts.tile([RPC, DIM], FR, tag="wo")

            def emit_late_consts():
                # SWDGE so these do not queue ahead of the x-tile loads on
                # the HWDGE rings; must still be emitted before any
                # consumer (Tile dependencies follow program order).
                nc.gpsimd.dma_start(out=ropec_s[:], in_=ropec[:])
                nc.gpsimd.dma_start(out=ropes_s[:], in_=ropes[:])
                for k in range(4):
                    nc.gpsimd.dma_start(out=mask_s[:, k * 512:(k + 1) * 512],
                                        in_=masks[k])
                nc.gpsimd.dma_start(out=wo_s[:], in_=wo[:])

            if use_pad_mask:
                pad_s = consts.tile([128, B * (T // 128)], F32, tag="padv")
                for b in range(B):
                    nc.sync.dma_start(
                        out=pad_s[:, b * 16:(b + 1) * 16], in_=padv[b])

            # ---- persistent activations (per-512-chunk tiles so phases
            # can overlap at chunk granularity) ----------------------------
            qrot = [persist.tile([RPC, 512], FR, tag=f"qrot{n}",
                                 name=f"qrot{n}") for n in range(NQC)]
            krot = [persist.tile([RPC, 512], FR, tag=f"krot{n}",
                                 name=f"krot{n}") for n in range(NQC)]
            ctxt = [persist.tile([RPC, 512], FR, tag=f"ctxt{n}",
                                 name=f"ctxt{n}") for n in range(NQC)]
            vaug = {}
            for b in range(B):
                va = persist.tile([128, HPC * 16 * 65], FR, tag=f"vaug{b}")
                nc.vector.memset(va[:].bitcast(F32), 1.0)
                vaug[b] = va

            def emit_a_chunk(n):
                """QKV projection + RoPE + V transpose for one 512-token
                chunk."""
                t0 = n * 512
                xt = [xpool.tile([128, 512], FR, tag="xt",
                                 name=f"xt{n}_{kc}") for kc in range(8)]
                for kc in range(8):
                    nc.sync.dma_start(
                        out=xt[kc][:],
                        in_=xT[kc * 128:(kc + 1) * 128, t0:t0 + 512])
                tloc = t0 % T

                def project(w_s, b_s, dst_raw):
                    ps = psA.tile([128, 512], F32, tag="proj")
                    for kc in range(8):
                        nc.tensor.matmul(ps[:],
                                         w_s[:, kc * 128:(kc + 1) * 128],
                                         xt[kc][:], start=(kc == 0),
                                         stop=(kc == 7))
                    nc.scalar.activation(dst_raw[:], ps[:], IDN, bias=b_s[:])

                def rope(raw, dst):
                    sw = psA.tile([128, 512], F32, tag="proj", name="sw")
                    nc.tensor.matmul(sw[:], swap_s[:], raw[:], start=True,
                                     stop=True)
                    nc.vector.tensor_mul(dst, raw[:],
                                         ropec_s[:, tloc:tloc + 512])
                    rtmp = ropetmp.tile([128, 512], F32, tag="rtmp")
                    nc.vector.tensor_mul(rtmp[:], sw[:],
                                         ropes_s[:, tloc:tloc + 512])
                    nc.vector.tensor_add(dst, dst, rtmp[:])

                qraw = qkvtmp.tile([128, 512], FR, tag="qraw")
                project(wq_s, bq_s, qraw)
                kraw = qkvtmp.tile([128, 512], FR, tag="kraw")
                project(wk_s, bk_s, kraw)
                rope(qraw, qrot[n][:, :])
                vraw = qkvtmp.tile([128, 512], FR, tag="vraw")
                project(wv_s, bv_s, vraw)
                rope(kraw, krot[n][:, :])
                b = t0 // T
                for m in range(4):
                    blk = (tloc + m * 128) // 128
                    tp = tps.tile([128, 128], FR, tag="tp")
                    nc.tensor.transpose(tp[:], vraw[:, m * 128:(m + 1) * 128],
                                        id_s[:])
                    dst = vaug[b].rearrange("p (h c) -> p h c", h=HPC)[
                        :, :, blk * 65:blk * 65 + 64]
                    src_ = tp[:].rearrange("p (h d) -> p h d", h=HPC)
                    nc.scalar.activation(dst, src_, CPY)

            def emit_b_pair(b, i):
                """Flash attention for one (batch, 512-query-block), BOTH
                heads interleaved with a 2-deep software pipeline so the PE
                always has an independent score matmul between a score and
                the PV that depends on its exp.  Scores are transposed
                [s,t]; PV is transposed too: ctx^T (+denominator row) =
                vaug^T @ exp(scores^T)."""
                tq0 = b * T + i * 512
                nj = 4 * (i + 1)
                cps = {}
                exs = {}
                for h in range(HPC):
                    cps[h] = ctxps.tile([65, 512], F32, tag="cps",
                                        name=f"cps{b}{h}{i}")

                def emit_sc(h, j):
                    hh = h * Dh
                    kchunk = (b * T + j * 128) // 512
                    koff = (j * 128) % 512
                    klhs = krot[kchunk][hh:hh + Dh, koff:koff + 128]
                    qrhs = qrot[4 * b + i][hh:hh + Dh, :]
                    sc = psA.tile([128, 512], F32, tag="proj",
                                  name=f"sc{b}{h}{i}{j}")
                    ex = exppool.tile([128, 512], FR, tag="ex")
                    kdiag = j - (nj - 4)
                    if kdiag < 0:
                        nc.tensor.matmul(sc[:], klhs, qrhs,
                                         start=True, stop=True)
                        nc.scalar.activation(ex[:], sc[:], EXP)
                    else:
                        # diagonal block: compute from the diagonal
                        # rightward; the diagonal 128-sub gets the
                        # triangular mask, the rest needs none
                        cols = slice(kdiag * 128, 512)
                        dcols = slice(kdiag * 128, (kdiag + 1) * 128)
                        nc.tensor.matmul(sc[:, cols], klhs, qrhs[:, cols],
                                         start=True, stop=True)
                        nc.scalar.activation(ex[:, cols], sc[:, cols], EXP)
                        nc.vector.tensor_mul(ex[:, dcols], ex[:, dcols],
                                             mask_s[:, 0:128])
                    if use_pad_mask:
                        nc.vector.tensor_scalar_mul(
                            ex[:], ex[:],
                            pad_s[:, b * 16 + j:b * 16 + j + 1])
                    exs[(h, j)] = ex

                def emit_pv(h, j):
                    va = vaug[b][:, h * (16 * 65):(h + 1) * (16 * 65)]
                    kdiag = max(0, j - (nj - 4))
                    cols = slice(kdiag * 128, 512)
                    nc.tensor.matmul(cps[h][:, cols],
                                     va[:, j * 65:j * 65 + 65],
                                     exs.pop((h, j))[:, cols],
                                     start=(j == 0), stop=(j == nj - 1))

                for j in range(nj + 3):
                    for h in range(HPC):
                        if j < nj:
                            emit_sc(h, j)
                        if j >= 3:
                            emit_pv(h, j - 3)
                # normalize rows 0..63 by denominator row 64 (DRAM-bounce
                # broadcast: engines cannot replicate a row across
                # partitions)
                for h in range(HPC):
                    hh = h * Dh
                    rec = normtmp.tile([1, 512], F32, tag="rec")
                    nc.vector.reciprocal(rec[:], cps[h][64:65, :])
                    scr = drampool.tile([1, 512], F32, tag="scr")
                    nc.sync.dma_start(out=scr[:], in_=rec[:])
                    bc = normtmp.tile([64, 512], F32, tag="bc")
                    nc.sync.dma_start(out=bc[:],
                                       in_=scr[:].partition_broadcast(64))
                    nc.vector.tensor_mul(ctxt[4 * b + i][hh:hh + Dh, :],
                                         cps[h][0:64, :], bc[:])

            def emit_c_chunk(n2):
                """Output projection (column-shard partial) for one
                512-token chunk."""
                for e in range(DIM // 128):
                    ps = psA.tile([128, 512], F32, tag="proj",
                                  name=f"op{e}_{n2}")
                    nc.tensor.matmul(ps[:], wo_s[:, e * 128:(e + 1) * 128],
                                     ctxt[n2][:, :], start=True, stop=True)
                    ev = outev.tile([128, 512], F32, tag="ev")
                    if e % 2 == 0:
                        nc.vector.tensor_copy(ev[:], ps[:])
                    else:
                        nc.scalar.activation(ev[:], ps[:], CPY)
                    nc.sync.dma_start(
                        out=outT[e * 128:(e + 1) * 128,
                                 n2 * 512:(n2 + 1) * 512],
                        in_=ev[:])

            for _rep in range(reps):
                # Interleaved emission: engines execute their streams in
                # program order, so phase overlap must be baked into the
                # order. b=1 projections interleave with b=0 attention;
                # output projection interleaves with b=1 attention.
                emit_late_consts()
                for i in range(4):
                    emit_a_chunk(i)
                    emit_b_pair(0, i)
                for i in range(4):
                    emit_a_chunk(4 + i)
                    emit_c_chunk(i)
                    if i > 0:
                        emit_c_chunk(3 + i)
                    emit_b_pair(1, i)
                emit_c_chunk(7)
    return nc


_NC_CACHE = {}


def _get_nc(use_pad_mask: bool, reps: int = 1):
    key = (use_pad_mask, reps)
    if key not in _NC_CACHE:
        _NC_CACHE[key] = _build_nc(use_pad_mask, reps)
    return _NC_CACHE[key]


def _host_inputs(x, attention_mask, qkv_w, qkv_b, out_w, use_pad_mask):
    """Build the 8 per-core input maps."""
    xT = np.ascontiguousarray(
        x.reshape(NT, DIM).T.astype(np.float32))            # [1024, 4096]
    C, S = _rope_tables()
    swapm = _swap_matrix()
    ident = np.eye(128, dtype=np.float32)
    masks = _tri_masks()
    perm = _deinterleave_perm()

    qkv_w = np.asarray(qkv_w, dtype=np.float32)
    qkv_b = np.asarray(qkv_b, dtype=np.float32)
    out_w = np.asarray(out_w, dtype=np.float32)

    in_maps = []
    for c in range(NCORES):
        heads = [HPC * c + h for h in range(HPC)]
        # q/k rows get the de-interleave permutation; q gets the 1/sqrt(Dh)
        qrows = np.concatenate([h * Dh + perm for h in heads])
        vrows = np.concatenate(
            [h * Dh + np.arange(Dh) for h in heads])
        wq_c = qkv_w[qrows, :] * SCALE                       # [128, 1024]
        wk_c = qkv_w[DIM + qrows, :]
        wv_c = qkv_w[2 * DIM + vrows, :]
        def pack_w(w_c):
            # SBUF layout [128, 1024]: row p, cols kc*128+m hold
            # W^T[kc*128+p, m] - one contiguous DMA per weight
            return np.ascontiguousarray(
                w_c.T.reshape(8, 128, RPC).transpose(1, 0, 2).reshape(
                    128, DIM))

        m = {
            "xT": xT,
            "wq": pack_w(wq_c),
            "wk": pack_w(wk_c),
            "wv": pack_w(wv_c),
            "bq": np.ascontiguousarray(
                (qkv_b[qrows] * SCALE).reshape(RPC, 1)),
            "bk": np.ascontiguousarray(qkv_b[DIM + qrows].reshape(RPC, 1)),
            "bv": np.ascontiguousarray(
                qkv_b[2 * DIM + vrows].reshape(RPC, 1)),
            "ropec": C,
            "ropes": S,
            "swapm": swapm,
            "ident": ident,
            "masks": masks,
            "wo": np.ascontiguousarray(
                out_w[:, c * RPC:(c + 1) * RPC].T),          # [128, 1024]
            "ones64": np.ones((1, Dh), dtype=np.float32),
        }
        if use_pad_mask:
            pad = np.asarray(attention_mask, dtype=np.float32)  # [B, T]
            m["padv"] = np.ascontiguousarray(
                pad.reshape(B, T // 128, 128).transpose(0, 2, 1))
        in_maps.append(m)
    return in_maps


def kernel(x, attention_mask, qkv_w, qkv_b, out_w, out_b):
    from concourse.bass_utils import run_bass_kernel_spmd

    use_pad_mask = not np.asarray(attention_mask).all()
    nc = _get_nc(use_pad_mask)
    in_maps = _host_inputs(x, attention_mask, qkv_w, qkv_b, out_w,
                           use_pad_mask)
    res = run_bass_kernel_spmd(nc, in_maps, list(range(NCORES)))
    acc = res.results[0]["outT"].astype(np.float32)
    for c in range(1, NCORES):
        acc = acc + res.results[c]["outT"]
    out = acc.T + np.asarray(out_b, dtype=np.float32)[None, :]
    return np.ascontiguousarray(out.reshape(B, T, DIM), dtype=np.float32)

